# revision 29
# baseline (speedup 1.0000x reference)
"""Trainium2 Bass kernel for ActivationHyperbolic (Poincare ball, relu activation).

Math (per row of x [N, 64], c scalar, s = sqrt(c)):
    xn  = max(||x||, 1e-7)
    arg = min(s*xn, 1 - 1e-7)
    u   = relu(x) * atanh(arg)/(s*xn)        (relu commutes with the
    un  = ||u|| = scale1 * ||relu(x)||        positive per-row scale)
    y   = u * tanh(s*un)/(s*un)
    out = y * min(maxnorm/||y||, 1),  maxnorm = (1-4e-3)/s

Everything collapses to out = relu(x) * total with per-row
    total = (1/(s*rn)) * min(tanh(z), 1-4e-3),   z = 0.5*d*s*g
    d = 2*atanh(a) = ln(1+a) - ln(1-a),  a = min(s*xn, 1-1e-7)
    g = rn/(s*xn),  rn = ||relu(x)||
(derivation: scale1*scale2 = invrn/s * tanh(z), maxnorm/rn = (1-eps)/(s*rn))

Row-reductions A = sum(x^2), B = sum(relu(x)^2) use DVE tensor_reduce
(1x, the unavoidable cost); everything else is spread so no engine
exceeds the reduce budget.

Engine split per [128, 4096] bf16 tile (8192 rows):
    DVE : reduce(A) + reduce(B) + r=relu(x) (4x TS) + 1/3 of rsq (2x TT)
          + chain arithmetic
    ACT : xsq=Square(x), 2/3 of rsq=Square(r), texp = broadcast-expand
          (total), chain transcendentals (all natural_log_exp table set)
    POOL: out = r * texp (2-input tensor_tensor)
    DMA : in 1 MiB bf16, out 1 MiB bf16

Sharding: pure data-parallel, rows split evenly across 8 NeuronCores.
I/O is bf16 on-device (host casts in, upcasts out) under the 2e-2
rel-err budget; measured global L2 error ~5e-3.
"""

import math
import sys

import numpy as np

for _p in ("/opt/trn_rl_repo",):
    if _p not in sys.path:
        sys.path.insert(0, _p)

import concourse.bass as bass
import concourse.tile as tile
from concourse import mybir
from concourse.bass_utils import run_bass_kernel_spmd

P = 128                      # SBUF partitions
D = 64                       # feature dim
NCORES = 8
N_TOTAL = 2097152
ROWS = N_TOTAL // NCORES     # 262144 rows per core
K = 64                       # row-groups per partition per tile
F = K * D                    # flat free dim per tile (4096)
TILE_ROWS = P * K            # 8192 rows per tile
NTILES = ROWS // TILE_ROWS   # 32 tiles per core
G = 8                        # tiles per chain group
W = G * K                    # chain width (512 row-groups)

BALL_EPS = 4e-3
ATANH_EPS = 1e-7

AF = mybir.ActivationFunctionType
ALU = mybir.AluOpType
AX = mybir.AxisListType
F32 = mybir.dt.float32
BF16 = mybir.dt.bfloat16


def _split_dma_waits(nc: bass.Bass) -> None:
    """Walrus can encode only ONE semaphore wait on a PSEUDO_DMA_DIRECT2D
    instruction (NEURON_ISA_TPB_EVENTS has a single wait slot). Tile may
    attach 2-3 waits to a DMA (slot-reuse WAR + queue WAW). Hoist all but
    one wait onto standalone event-semaphore instructions executed by the
    same engine immediately before the DMA — same semantics, encodable."""
    for f in nc.m.functions:
        for bb in f.blocks:
            new_insts = []
            for ins in bb.instructions:
                si = ins.sync_info
                if (
                    si is not None
                    and si.on_wait
                    and len(si.on_wait) > 1
                    and not isinstance(ins, mybir.InstEventSemaphore)
                ):
                    waits = list(si.on_wait)
                    for wsub in waits[:-1]:
                        wi = mybir.InstEventSemaphore(
                            name=f"I-dmawait-{nc.next_id()}",
                            ins=[],
                            outs=[],
                            engine=ins.engine,
                        )
                        wi.sync_info = mybir.SyncInfo(
                            on_wait=[wsub], on_update=[]
                        )
                        new_insts.append(wi)
                    ins.sync_info = mybir.SyncInfo(
                        on_wait=[waits[-1]], on_update=list(si.on_update)
                    )
                new_insts.append(ins)
            bb.instructions[:] = new_insts


def _build(c_val: float) -> bass.Bass:
    s = math.sqrt(c_val)
    ln_s = math.log(s)
    m = 1.0 - ATANH_EPS
    ceps = 1.0 - BALL_EPS

    nc = bass.Bass()

    # Register activation bias constants ([128,1] const APs; only 0.0/1.0
    # are pre-registered by Bass).
    def _register_const(value: float):
        if (F32, value) in nc.const_aps.aps:
            return
        t = nc.alloc_sbuf_tensor(f"const-f32-{value}", [128, 1], F32)
        nc.gpsimd.memset(t.ap(), value)
        nc.const_aps.aps[(F32, value)] = t.ap()

    for v in (ln_s, -ln_s, m, 1.0 + m, 1.0 - m, -1e-20, 1e-20, ceps, -ceps):
        _register_const(float(v))
    nc.all_engine_barrier()

    x = nc.declare_dram_parameter("x", [ROWS, D], BF16, isOutput=False)
    out = nc.declare_dram_parameter("out", [ROWS, D], BF16, isOutput=True)
    xr = x[:].rearrange("(t p k) d -> t p (k d)", p=P, k=K)
    outr = out[:].rearrange("(t p k) d -> t p (k d)", p=P, k=K)

    with tile.TileContext(nc, pool_alloc_mode="queue") as tc:
        with (
            tc.tile_pool(name="xin", bufs=3) as xin_pool,
            tc.tile_pool(name="rpool", bufs=G + 3) as r_pool,
            tc.tile_pool(name="sq", bufs=2) as sq_pool,
            tc.tile_pool(name="stats", bufs=2) as stats_pool,
            tc.tile_pool(name="chain", bufs=1) as chain_pool,
            tc.tile_pool(name="opool", bufs=3) as o_pool,
        ):
            def stream_tile(t, j, A, B):
                xt = xin_pool.tile([P, F], BF16, tag="x", name="x")
                nc.sync.dma_start(out=xt[:], in_=xr[t])
                # xsq = x^2 (ACT; DVE for tile 0 so DVE starts immediately)
                xsq = sq_pool.tile([P, F], BF16, tag="xsq", name="xsq")
                if t == 0:
                    nc.vector.tensor_tensor(xsq[:], xt[:], xt[:], ALU.mult)
                else:
                    nc.scalar.activation(xsq[:], xt[:], AF.Square)
                nc.vector.tensor_reduce(
                    A[:, j * K : (j + 1) * K],
                    xsq[:].rearrange("p (k d) -> p k d", d=D),
                    AX.X,
                    ALU.add,
                )
                # r = relu(x) (ACT), resident until output
                r = r_pool.tile([P, F], BF16, tag="r", name="r")
                nc.scalar.activation(r[:], xt[:], AF.Relu)
                # rsq = r^2: DVE TT 14/32, POOL TT 2/32, ACT Square 16/32
                rsq = sq_pool.tile([P, F], BF16, tag="rsq", name="rsq")
                if t % 16 < 7:
                    nc.vector.tensor_tensor(rsq[:], r[:], r[:], ALU.mult)
                elif t % 16 == 7:
                    nc.gpsimd.tensor_tensor(rsq[:], r[:], r[:], ALU.mult)
                else:
                    nc.scalar.activation(rsq[:], r[:], AF.Square)
                nc.vector.tensor_reduce(
                    B[:, j * K : (j + 1) * K],
                    rsq[:].rearrange("p (k d) -> p k d", d=D),
                    AX.X,
                    ALU.add,
                )
                return (t, r)

            def chain_group(A, B, wg=W):
                # --- per-row chain (exp/ln only: one ACT table set) -------
                # ACT does the transcendentals; POOL does the tensor-tensor
                # arithmetic (keeps the serial zig-zag off DVE's in-order
                # queue). Two half-width chunks interleaved so ACT and POOL
                # overlap across chunks instead of stalling on each other.
                H = wg // 2
                total = chain_pool.tile([P, wg], F32, tag="total", name="total")

                def chunk_tiles(tag):
                    return [
                        chain_pool.tile([P, H], F32, tag=f"{tag}{h}", name=f"{tag}{h}")
                        for h in (0, 1)
                    ]

                L = chunk_tiles("L"); q1 = chunk_tiles("q1")
                q2 = chunk_tiles("q2"); q3 = chunk_tiles("q3")
                M = chunk_tiles("M"); g = chunk_tiles("g"); iv = chunk_tiles("iv")
                E = chunk_tiles("E")
                Ah = [A[:, :H], A[:, H : 2 * H]]
                Bh = [B[:, :H], B[:, H : 2 * H]]
                Th = [total[:, :H], total[:, H : 2 * H]]

                def each(fn):
                    for h in (0, 1):
                        fn(h)

                each(lambda h: nc.scalar.activation(L[h][:], Ah[h], AF.Ln))
                each(lambda h: nc.scalar.activation(q1[h][:], L[h][:], AF.Exp, scale=0.5, bias=ln_s))    # argu
                each(lambda h: nc.scalar.activation(q1[h][:], q1[h][:], AF.Relu, scale=-1.0, bias=m))    # w
                each(lambda h: nc.scalar.activation(q2[h][:], q1[h][:], AF.Ln, scale=-1.0, bias=1.0 + m))  # ln(1+a)
                each(lambda h: nc.scalar.activation(q1[h][:], q1[h][:], AF.Ln, scale=1.0, bias=1.0 - m))   # ln(1-a)
                each(lambda h: nc.scalar.activation(M[h][:], Bh[h], AF.Relu, bias=-1e-20))
                each(lambda h: nc.scalar.activation(M[h][:], M[h][:], AF.Ln, bias=1e-20))
                each(lambda h: nc.gpsimd.tensor_tensor(q3[h][:], M[h][:], L[h][:], ALU.subtract))        # h = M-L
                each(lambda h: nc.scalar.activation(g[h][:], q3[h][:], AF.Exp, scale=0.5, bias=-ln_s))   # rn/(s*xn)
                each(lambda h: nc.scalar.activation(iv[h][:], M[h][:], AF.Exp, scale=-0.5, bias=-ln_s))  # 1/(s*rn)
                each(lambda h: nc.gpsimd.tensor_tensor(q2[h][:], q2[h][:], q1[h][:], ALU.subtract))      # d = 2atanh
                each(lambda h: nc.gpsimd.tensor_tensor(q3[h][:], q2[h][:], g[h][:], ALU.mult))           # z' = d*g
                each(lambda h: nc.scalar.activation(E[h][:], q3[h][:], AF.Exp, scale=s))                 # e^{2z}=e^{s*z'}
                each(lambda h: nc.scalar.activation(q1[h][:], E[h][:], AF.Ln, bias=1.0))                 # ln(E+1)
                each(lambda h: nc.scalar.activation(q1[h][:], q1[h][:], AF.Exp, scale=-1.0))             # Q=1/(E+1)
                each(lambda h: nc.gpsimd.tensor_tensor(q3[h][:], E[h][:], q1[h][:], ALU.mult))           # E*Q
                each(lambda h: nc.gpsimd.tensor_tensor(q2[h][:], q3[h][:], q1[h][:], ALU.subtract))      # tanh=(E-1)Q
                # min(tanh, ceps) = tanh - relu(tanh - ceps)  (min is not
                # POOL-legal and we keep the chain off DVE's in-order queue)
                each(lambda h: nc.scalar.activation(q3[h][:], q2[h][:], AF.Relu, bias=-ceps))
                each(lambda h: nc.gpsimd.tensor_tensor(q1[h][:], q2[h][:], q3[h][:], ALU.subtract))
                each(lambda h: nc.gpsimd.tensor_tensor(Th[h], q1[h][:], iv[h][:], ALU.mult))             # total
                return total

            def outmul_tile(total, j, tr):
                t, r = tr
                tot3 = total[:].rearrange("p (g k) -> p g k", k=K)
                tb = tot3[:, j, :].to_broadcast((P, K, D))
                ot = o_pool.tile([P, F], BF16, tag="o", name="o")
                o3 = ot[:].rearrange("p (k d) -> p k d", d=D)
                r3 = r[:].rearrange("p (k d) -> p k d", d=D)
                nc.gpsimd.tensor_tensor(o3, r3, tb, ALU.mult)
                nc.sync.dma_start(out=outr[t], in_=ot[:])

            # Software pipeline. Per phase: first stream-tile of group g,
            # then the previous group's chain (its inputs are ready, and
            # emitting it early keeps it off the back of ACT's queue),
            # then the remaining stream tiles interleaved with the
            # previous group's output multiplies (frees r slots steadily).
            # The last two groups are half-size so the pipeline drain
            # (final chain + outmuls after the last reduce) is short.
            group_sizes = [8, 8, 8, 4, 2, 2]
            assert sum(group_sizes) == NTILES
            starts = [sum(group_sizes[:i]) for i in range(len(group_sizes))]
            pending = None
            for gs, st in zip(group_sizes, starts):
                tiles = list(range(st, st + gs))
                A = stats_pool.tile([P, W], F32, tag="A", name="A")
                B = stats_pool.tile([P, W], F32, tag="B", name="B")
                rs = [stream_tile(tiles[0], 0, A, B)]
                totalp = None
                if pending is not None:
                    Ap, Bp, rsp = pending
                    totalp = chain_group(Ap, Bp, wg=len(rsp) * K)
                for j in range(1, gs):
                    rs.append(stream_tile(tiles[j], j, A, B))
                    if totalp is not None and j - 1 < len(rsp):
                        outmul_tile(totalp, j - 1, rsp[j - 1])
                if totalp is not None:
                    for jj in range(max(0, gs - 1), len(rsp)):
                        outmul_tile(totalp, jj, rsp[jj])
                pending = (A, B, rs)
            Ap, Bp, rsp = pending
            totalp = chain_group(Ap, Bp, wg=len(rsp) * K)
            for j in range(len(rsp)):
                outmul_tile(totalp, j, rsp[j])

    _split_dma_waits(nc)
    return nc


_BUILD_CACHE: dict[float, bass.Bass] = {}


def _run(x: np.ndarray, c: np.ndarray, trace: bool = False):
    import ml_dtypes

    assert x.shape == (N_TOTAL, D), x.shape
    x = np.ascontiguousarray(x, dtype=np.float32).astype(ml_dtypes.bfloat16)
    c_val = float(np.asarray(c).reshape(-1)[0])
    nc = _BUILD_CACHE.get(c_val)
    if nc is None:
        nc = _build(c_val)
        _BUILD_CACHE[c_val] = nc
    shards = np.split(x, NCORES, axis=0)
    in_maps = [{"x": sh} for sh in shards]
    res = run_bass_kernel_spmd(
        nc, in_maps, core_ids=list(range(NCORES)), trace=trace
    )
    out = np.concatenate(
        [np.asarray(res.results[i]["out"]) for i in range(NCORES)], axis=0
    ).astype(np.float32)
    return out, res


def kernel(x: np.ndarray, c: np.ndarray) -> np.ndarray:
    out, _ = _run(x, c, trace=False)
    return out
